# revision 2
# baseline (speedup 1.0000x reference)
"""Multi-head attention (B=4, S=2048, H=8, d_head=16) on 8 trn2 cores.

One head per core. Transposed-scores dataflow, ScalarE-exp-bound design:

  S^T[k,q] = matmul(lhsT=kT[49,128], rhs=qT[49,512])  bf16, f32 PSUM
      rows: 3 significant cross terms of the hi/lo bf16 split of Q and
      (4*K)  (hh, lh, hl -> ~16-bit logit precision), plus one bias row
      (q-side ones) carrying -75 (global shift) or -1e30 (length mask),
      so PSUM holds the finished exp argument.
  exp on ScalarE only (nothing else on TRN2 can exp). Units are grouped
  in patterns [direct, staged, staged]:
      direct: exp reads the [128,1024] PSUM tiles straight (2x 1038ns)
      staged: DVE copies PSUM->SBUF f32; one [128, 4096] exp covers two
      units, amortizing ScalarE's per-instruction access penalty. The
      staged exp and the AV matmuls lag one pattern so ScalarE never
      stalls on the copies.
  AV: out[34,512] += matmul(lhsT=vo[128,34] bf16, rhs=pt bf16) PSUM-
      accumulated over k-tiles; vo cols = [V_hi |1| V_lo |0]. The two
      q-halves pack into one PSUM tile at partitions 0 and 64 (f32r
      would be Ldweights-free but its s3d3 mode forbids dst partition
      64), halving output-copy columns. Host sums hi+lo and divides by
      the ones row.

k-tiles beyond ceil(seq_len/128) are skipped (baked per batch). Batches
are processed in descending k-tile count so the tail batch is cheap.
"""

import ml_dtypes
import numpy as np

import concourse.bass as bass
import concourse.tile as tile
from concourse import bacc, mybir
from concourse.bass_utils import run_bass_kernel_spmd

B = 4
S = 2048
H = 8
DH = 16
KT_TILE = 128
C_SHIFT = 75.0
NEG_BIG = -1.0e30
F32 = mybir.dt.float32
F32R = mybir.dt.float32r
BF16 = mybir.dt.bfloat16

NROW = 49  # 3*16 cross-term rows + 1 bias row

_cache = {}


PLAN = "DS2"
PRO_D = 3
TAPER = 1
SPLITQ = 0
TAIL_D = 6
NWARM = 4
ST_BUFS = 3
OT_BUFS = 1
AV_LAG = 2


def _plan_patterns(n_units):
    """Repeating [direct, staged-pair] patterns; trailing units direct."""
    pats = []
    u = 0
    if PLAN == "ALLD":
        return [(i, []) for i in range(n_units)]
    while u < PRO_D:
        pats.append((u, [])); u += 1
    while u < n_units:
        rem = n_units - u
        if rem <= TAIL_D:
            pats.append((u, [])); u += 1
        elif PLAN == "DS2" and rem >= 3:
            pats.append((u, [u + 1, u + 2])); u += 3
        elif PLAN == "DDS2" and rem >= 4:
            pats.append((u, [])); pats.append((u + 1, [u + 2, u + 3])); u += 4
        elif PLAN == "DS3" and rem >= 4:
            pats.append((u, [u + 1, u + 2, u + 3])); u += 4
        else:
            pats.append((u, [])); u += 1
    return pats


def _build(nbs_proc, order):
    nb_total = sum(nbs_proc)
    kt_cols = nb_total * KT_TILE

    nc = bacc.Bacc(
        "TRN2",
        target_bir_lowering=False,
        debug=False,
        num_devices=8,
    )

    qT_d = nc.dram_tensor("qT", [B, NROW, S], BF16, kind="ExternalInput").ap()
    kT_d = nc.dram_tensor("kT", [NROW, kt_cols], BF16, kind="ExternalInput").ap()
    vo_d = nc.dram_tensor("vo", [128, nb_total * 34], BF16, kind="ExternalInput").ap()
    out_d = nc.dram_tensor("outT", [B, 2 * DH + 2, S], F32, kind="ExternalOutput").ap()

    units = []
    for pb in range(B):
        off = sum(nbs_proc[:pb])
        for kt in range(nbs_proc[pb]):
            units.append((pb, kt, off + kt, kt == 0, kt == nbs_proc[pb] - 1))
    pats = _plan_patterns(len(units))
    last_pb = units[-1][0]

    with tile.TileContext(nc) as tc:
        with (
            tc.tile_pool(name="const", bufs=1) as const,
            tc.tile_pool(name="st", bufs=ST_BUFS, space="PSUM") as stpool,
            tc.tile_pool(name="ot", bufs=OT_BUFS, space="PSUM") as otpool,
            tc.tile_pool(name="ss", bufs=2) as sspool,
            tc.tile_pool(name="pts", bufs=3) as ptspool,
            tc.tile_pool(name="ptd", bufs=4) as ptdpool,
            tc.tile_pool(name="ob", bufs=2) as obpool,
        ):
            q_tiles = []
            for pb in range(B):
                qt = const.tile([NROW, S], BF16, tag=f"qT{pb}")
                q_tiles.append(qt)
            kT_t = const.tile([NROW, kt_cols], BF16, tag="kT")
            vo_t = const.tile([128, nb_total * 34], BF16, tag="vo")

            # Critical-path DMAs first.
            nc.sync.dma_start(kT_t[:, 0:128], kT_d[:, 0:128])
            nc.scalar.dma_start(q_tiles[0][:, 0:512], qT_d[0][:, 0:512])
            nc.sync.dma_start(q_tiles[0][:, 512:1024], qT_d[0][:, 512:1024])
            nc.scalar.dma_start(q_tiles[0][:, 1024:2048], qT_d[0][:, 1024:2048])

            # Prefetch the exp table; warm the PE clock with tiny matmuls.
            warm = const.tile([1, 1], F32, tag="warm")
            nc.vector.memset(warm[:], 0.0)
            nc.scalar.activation(warm[:], warm[:], mybir.ActivationFunctionType.Exp)
            pewarm = const.tile([NROW, 128], BF16, tag="pewarm")
            nc.vector.memset(pewarm[:], 0.0)
            if NWARM:
                st_w = stpool.tile([128, 1024], F32, tag="st")
                for j in range(NWARM):
                    nc.tensor.matmul(
                        st_w[:, 128 * j:128 * (j + 1)],
                        pewarm[:],
                        pewarm[:],
                        start=True,
                        stop=True,
                    )

            # Bulk loads via gpsimd SWDGE, first-needed first.
            if nbs_proc[0] > 1:
                nc.gpsimd.dma_start(
                    kT_t[:, 128:nbs_proc[0] * 128], kT_d[:, 128:nbs_proc[0] * 128]
                )
            nc.gpsimd.dma_start(
                vo_t[:, 0:nbs_proc[0] * 34], vo_d[:, 0:nbs_proc[0] * 34]
            )
            for pb in range(1, B):
                off = sum(nbs_proc[:pb])
                nb = nbs_proc[pb]
                nc.gpsimd.dma_start(q_tiles[pb][:], qT_d[pb])
                nc.gpsimd.dma_start(
                    kT_t[:, off * 128:(off + nb) * 128],
                    kT_d[:, off * 128:(off + nb) * 128],
                )
                nc.gpsimd.dma_start(
                    vo_t[:, off * 34:(off + nb) * 34],
                    vo_d[:, off * 34:(off + nb) * 34],
                )

            av_next = [0]
            ots = {}       # proc_batch -> ot tile
            pt_of = {}     # unit -> (tile, col_offset)
            ss_of = {}     # pattern index -> (ss tile, [staged units])

            def emit_st(u, staged, ss, sscol):
                pb, kt, t, first, last = units[u]
                for half in range(2):
                    st = stpool.tile([128, 1024], F32, tag="st")
                    for i in range(2):
                        qs = 1024 * half + 512 * i
                        nc.tensor.matmul(
                            st[:, 512 * i:512 * (i + 1)],
                            kT_t[:, t * 128:(t + 1) * 128],
                            q_tiles[pb][:, qs:qs + 512],
                            start=True,
                            stop=True,
                        )
                    if staged:
                        nc.vector.tensor_copy(
                            ss[:, sscol + 1024 * half:sscol + 1024 * (half + 1)],
                            st[:],
                        )
                    else:
                        if half == 0:
                            ptd = ptdpool.tile([128, 2048], BF16, tag="ptd")
                            pt_of[u] = (ptd, 0)
                        ptd = pt_of[u][0]
                        nc.scalar.activation(
                            ptd[:, 1024 * half:1024 * (half + 1)],
                            st[:],
                            mybir.ActivationFunctionType.Exp,
                        )

            def emit_bigexp(p):
                ss, sus = ss_of.pop(p)
                pts = ptspool.tile([128, 2048 * len(sus)], BF16, tag="pts")
                nc.scalar.activation(
                    pts[:],
                    ss[:],
                    mybir.ActivationFunctionType.Exp,
                )
                for j, u in enumerate(sus):
                    pt_of[u] = (pts, j * 2048)

            def emit_av_unit(u):
                pb, kt, t, first, last = units[u]
                if first:
                    ot_new = otpool.tile([98, 1024], F32, tag="ot")
                    ots[pb] = ot_new
                ot = ots[pb]
                pt, coff = pt_of.pop(u)
                for half in range(2):
                    prow = 64 * half
                    for i in range(2):
                        qs = coff + 1024 * half + 512 * i
                        nc.tensor.matmul(
                            ot[prow:prow + 34, 512 * i:512 * (i + 1)],
                            vo_t[:, t * 34:(t + 1) * 34],
                            pt[:, qs:qs + 512],
                            start=first,
                            stop=last,
                        )
                if last:
                    ob = obpool.tile([98, 1024], F32, tag="ob")
                    b = order[pb]
                    if pb == last_pb:
                        # Tail batch: split copy across DVE+Act, 4 DMAs on
                        # 2 queues so the drain pipeline overlaps.
                        nc.vector.tensor_copy(ob[:, 0:512], ot[:, 0:512])
                        nc.sync.dma_start(out_d[b][:, 0:512], ob[0:34, 0:512])
                        nc.scalar.copy(ob[:, 512:1024], ot[:, 512:1024])
                        nc.scalar.dma_start(
                            out_d[b][:, 1024:1536], ob[64:98, 0:512]
                        )
                        nc.sync.dma_start(out_d[b][:, 512:1024], ob[0:34, 512:1024])
                        nc.scalar.dma_start(
                            out_d[b][:, 1536:2048], ob[64:98, 512:1024]
                        )
                    else:
                        nc.vector.tensor_copy(ob[:], ot[:])
                        nc.sync.dma_start(out_d[b][:, 0:1024], ob[0:34, :])
                        nc.scalar.dma_start(out_d[b][:, 1024:2048], ob[64:98, :])

            def emit_av_pat(p):
                pd, psus = pats[p]
                emit_av_unit(pd)
                for u in psus:
                    emit_av_unit(u)

            for p, (du, sus) in enumerate(pats):
                emit_st(du, False, None, 0)
                if sus:
                    ss = sspool.tile([128, 2048 * len(sus)], F32, tag="ss")
                    ss_of[p] = (ss, sus)
                    for j, u in enumerate(sus):
                        emit_st(u, True, ss, j * 2048)
                if p > 0 and pats[p - 1][1]:
                    emit_bigexp(p - 1)
                while av_next[0] <= p - AV_LAG or (
                    av_next[0] == p - 1 and not pats[av_next[0]][1]
                ):
                    emit_av_pat(av_next[0])
                    av_next[0] += 1
            P = len(pats)
            if pats[P - 1][1]:
                emit_bigexp(P - 1)
            while av_next[0] < P:
                emit_av_pat(av_next[0])
                av_next[0] += 1

    nc.compile()
    return nc


def kernel(key_and_value, query, seq_len):
    key_and_value = np.asarray(key_and_value, dtype=np.float32)
    query = np.asarray(query, dtype=np.float32)
    sl = np.asarray(seq_len).reshape(-1).astype(np.int64)

    nbs = tuple(int(-(-int(s) // KT_TILE)) for s in sl)
    order = tuple(sorted(range(B), key=lambda b: -nbs[b]))
    nbs_proc = tuple(nbs[b] for b in order)

    key = (nbs_proc, order)
    if key not in _cache:
        _cache[key] = _build(nbs_proc, order)
    nc = _cache[key]

    k_all = key_and_value[:, :, :128] * 4.0   # fold sqrt(d_head) scale
    v_all = key_and_value[:, :, 128:]

    bf16 = ml_dtypes.bfloat16

    def hilo(x):
        hi = x.astype(bf16)
        lo = (x - hi.astype(np.float32)).astype(bf16)
        return hi, lo

    q_all_t = query.transpose(0, 2, 1)        # [B, 128, S]
    qhi_a, qlo_a = hilo(q_all_t)
    khi_a, klo_a = hilo(k_all)                # [B, S, 128]

    in_maps = []
    for h in range(H):
        c0 = h * DH
        qT = np.empty((B, NROW, S), dtype=bf16)
        kT_chunks = []
        vo_chunks = []
        for pb, b in enumerate(order):
            qT[pb, 0:DH] = qhi_a[b, c0:c0 + DH]
            qT[pb, DH:2 * DH] = qlo_a[b, c0:c0 + DH]
            qT[pb, 2 * DH:3 * DH] = qhi_a[b, c0:c0 + DH]
            qT[pb, 3 * DH] = np.float32(1.0)

            nrowk = nbs[b] * 128
            kc = np.empty((NROW, nrowk), dtype=bf16)
            kc[0:DH] = khi_a[b, :nrowk, c0:c0 + DH].T
            kc[DH:2 * DH] = khi_a[b, :nrowk, c0:c0 + DH].T
            kc[2 * DH:3 * DH] = klo_a[b, :nrowk, c0:c0 + DH].T
            karr = np.arange(nrowk)
            lam = np.where(karr < sl[b], np.float32(-C_SHIFT), np.float32(NEG_BIG))
            kc[3 * DH] = lam.astype(bf16)
            kT_chunks.append(kc)

            vb = v_all[b, :nrowk, c0:c0 + DH].reshape(nbs[b], 128, DH)
            vhi = vb.astype(bf16)
            vlo = (vb - vhi.astype(np.float32)).astype(bf16)
            vo_b = np.concatenate(
                [
                    vhi,
                    np.ones((nbs[b], 128, 1), dtype=bf16),
                    vlo,
                    np.zeros((nbs[b], 128, 1), dtype=bf16),
                ],
                axis=2,
            )  # [nb, 128, 34]
            vo_chunks.append(vo_b.transpose(1, 0, 2).reshape(128, nbs[b] * 34))
        kT = np.ascontiguousarray(np.concatenate(kT_chunks, axis=1))
        vo = np.ascontiguousarray(np.concatenate(vo_chunks, axis=1))
        in_maps.append({
            "qT": np.ascontiguousarray(qT),
            "kT": kT,
            "vo": vo,
        })

    import os

    trace = bool(os.environ.get("ATTN_TRACE"))
    kw = {}
    if trace:
        kw = dict(
            trace=True,
            tmpdir=os.environ.get("ATTN_TRACE_DIR") or None,
            trace_cores=[0],
        )
    res = run_bass_kernel_spmd(nc, in_maps, core_ids=list(range(H)), **kw)
    if trace and res.exec_time_ns is not None:
        print(f"HW exec time: {res.exec_time_ns} ns")
        kernel.last_exec_time_ns = res.exec_time_ns

    out = np.empty((B, S, H * DH), dtype=np.float32)
    for h in range(H):
        o = res.results[h]["outT"]  # [4, 34, 2048]
        num = o[:, :DH, :] + o[:, DH + 1:2 * DH + 1, :]
        den = o[:, DH:DH + 1, :] + o[:, 2 * DH + 1:2 * DH + 2, :]
        out[:, :, h * DH:(h + 1) * DH] = (num / den).transpose(0, 2, 1)
    return out
